# revision 1
# baseline (speedup 1.0000x reference)
"""Trainium2 Bass kernel for nn_CrossGraphConvolution.

Math (per batch b):
    tg  = l2norm(target_g[b], axis=C);  inp_n = l2norm(input[b], axis=C)
    adj = softmax(tg^T @ inp_n, axis=n)            # [M, N] cosine attention
    agg = adj @ input[b]^T                         # [M, C]
    out = BN(LeakyReLU(agg @ W)) permuted to [OUT, M]

Kernel strategy (one batch per NeuronCore, 8 cores, no collectives):
  * Flash-attention style fusion in the TRANSPOSED orientation: S^T tiles
    [n x m] come straight out of the PE (lhsT = normalized input block,
    rhs = normalized target), so exp(S^T) is already the P^T layout the
    aggregation matmul needs -- the 4096x4096 attention matrix is never
    transposed or spilled to HBM.
  * Softmax skips max-subtraction (cosines are in [-1,1], exp is safe).
    The 1/rowsum division is deferred to the very end: rowsum comes from
    a ones-stationary matmul over P^T, and LeakyReLU commutes with
    positive per-column scaling.
  * The [C,OUT] projection W is folded into the input before aggregation
    (XWT[n,o] = sum_c x[c,n] W[c,o]), so aggregation directly produces
    the [OUT, m] output block.
  * x/g ship as bf16 (the matmuls are bf16 anyway); the output ships as
    bf16 *delta* (pre-BN-shift) and the f32 channel shift is added on
    the host, so the small dynamic range keeps full precision.
"""

import sys

import numpy as np

if "/opt/trn_rl_repo" not in sys.path:
    sys.path.insert(0, "/opt/trn_rl_repo")

B, C, N, M, OUT = 8, 128, 4096, 4096, 128
NJ = N // 128          # n chunks (partition blocks of P^T)
MGS = 1024             # m-group width (PSUM accumulator free size)
NMG = M // MGS
EPS_BN = 1e-5
NEG_SLOPE = 0.01


def _apply_bir_passes():
    """Post-process the serialized BIR:

    1. Drop redundant Ldweights: a Ldweights identical to the previous
       one on the same engine (with no other Ldweights between) reloads
       the same stationary operand -- the PE keeps it, so it can go.
    2. Wait legalization: this walrus build rejects instructions
       carrying more than one sync wait.  Hoist all but one wait onto
       single-wait NoOps inserted immediately before the instruction on
       the same engine (same execution point, no reordering)."""
    import json

    import concourse.bass as bass

    if getattr(bass.Bass, "_bir_passes_applied", False):
        return
    orig = bass.Bass.to_json_bytes

    def patched(self):
        bir = json.loads(orig(self))
        for fn in bir.get("functions", []):
            for blk in fn.get("blocks", []):
                insts = blk.get("instructions", [])
                # pass 1: Ldweights dedup
                last_ldw = {}
                kept = []
                for ins in insts:
                    if ins.get("opcode") == "Ldweights":
                        eng = ins.get("engine")
                        key = json.dumps(
                            [
                                ins.get("ins"),
                                ins.get("perf_mode"),
                                ins.get("is_transpose"),
                                ins.get("tile_position"),
                            ],
                            sort_keys=True,
                        )
                        ow = (ins.get("sync_info") or {}).get("on_wait") or []
                        upd = (ins.get("sync_info") or {}).get("on_update") or []
                        if last_ldw.get(eng) == key and not upd:
                            if ow:
                                kept.append(
                                    {
                                        "debug": ins.get("debug", 0),
                                        "engine": eng,
                                        "ins": [],
                                        "name": ins["name"] + "-dedup",
                                        "opcode": "NoOp",
                                        "outs": [],
                                        "sync_info": {
                                            "on_update": [],
                                            "on_wait": ow,
                                        },
                                    }
                                )
                            continue
                        last_ldw[eng] = key
                    kept.append(ins)
                # pass 2: wait legalization
                new_insts = []
                for ins in kept:
                    si = ins.get("sync_info")
                    ow = (si or {}).get("on_wait") or []
                    if len(ow) > 1:
                        for k, w in enumerate(ow[:-1]):
                            new_insts.append(
                                {
                                    "debug": ins.get("debug", 0),
                                    "engine": ins["engine"],
                                    "ins": [],
                                    "name": f"{ins['name']}-w{k}",
                                    "opcode": "NoOp",
                                    "outs": [],
                                    "sync_info": {
                                        "on_update": [],
                                        "on_wait": [w],
                                    },
                                }
                            )
                        si["on_wait"] = [ow[-1]]
                    new_insts.append(ins)
                blk["instructions"] = new_insts
        return json.dumps(bir).encode()

    bass.Bass.to_json_bytes = patched
    bass.Bass._bir_passes_applied = True


# backwards-compat alias used by the dev test scripts
_apply_bir_wait_legalizer = _apply_bir_passes


def _bcast(ap, parts):
    """Partition-stride-0 view of a [1, ...] DRAM AP, for DMA broadcast."""
    import concourse.bass as bass

    return bass.AP(
        tensor=ap.tensor,
        offset=ap.offset,
        ap=[[0, parts]] + [list(d) for d in ap.ap[1:]],
    )


def build_nc(repeats: int = 1, rows_mode: str = "mm", in_dtype: str = "bf16"):
    import concourse.bass as bass
    import concourse.tile as tile
    from concourse import mybir

    _apply_bir_passes()

    f32 = mybir.dt.float32
    bf16 = mybir.dt.bfloat16
    in_dt = bf16 if in_dtype == "bf16" else mybir.dt.float8e3
    ALU = mybir.AluOpType
    ACTF = mybir.ActivationFunctionType

    nc = bass.Bass("TRN2")
    x_d = nc.dram_tensor("x", [C, N], in_dt, kind="ExternalInput")
    g_d = nc.dram_tensor("g", [C, M], in_dt, kind="ExternalInput")
    w_d = nc.dram_tensor("w", [C, OUT], f32, kind="ExternalInput")
    a_d = nc.dram_tensor("a_sc", [OUT, 1], f32, kind="ExternalInput")
    y_d = nc.dram_tensor("y", [OUT, M], bf16, kind="ExternalOutput")

    with tile.TileContext(nc) as tc:
        with (
            tc.tile_pool(name="const", bufs=1) as const,
            tc.tile_pool(name="sb", bufs=1) as sb,
        ):
            # ---- constants / params ----
            ones_bf = const.tile([128, 1], bf16, tag="ones", name="ones_bf")
            nc.vector.memset(ones_bf, 1.0)
            zero_b = const.tile([128, 1], f32, tag="zero_b", name="zero_b")
            nc.vector.memset(zero_b, 0.0)

            w_sb = const.tile([C, OUT], f32, tag="w", name="w_sb")
            nc.gpsimd.dma_start(out=w_sb, in_=w_d[:])
            w_bf = const.tile([C, OUT], bf16, tag="wbf", name="w_bf")
            nc.vector.tensor_copy(w_bf, w_sb)
            a_sc = const.tile([OUT, 1], f32, tag="a_sc", name="a_sc")
            nc.gpsimd.dma_start(out=a_sc, in_=a_d[:])

            # ---- inputs (bf16) ----
            x_sb = sb.tile([C, N], in_dt, tag="x", name="x_sb")
            g_sb = sb.tile([C, M], in_dt, tag="g", name="g_sb")
            nc.gpsimd.dma_start(out=x_sb, in_=x_d[:])
            nc.gpsimd.dma_start(out=g_sb, in_=g_d[:])

            xn_bf = sb.tile([C, N], bf16, tag="xn", name="xn_bf")
            gn_bf = sb.tile([C, M], bf16, tag="gn", name="gn_bf")
            xwt = sb.tile([128, NJ * OUT], bf16, tag="xwt", name="xwt")

            for _rep in range(repeats):
                # ---- phase A: column l2 norms + normalization ----
                with (
                    tc.tile_pool(name="pa_ps", bufs=1, space="PSUM") as pa_ps,
                    tc.tile_pool(name="pa_ps2", bufs=2, space="PSUM") as pa_ps2,
                    tc.tile_pool(name="pa_sb", bufs=2) as pa_sb,
                    tc.tile_pool(name="pa_sb1", bufs=1) as pa_sb1,
                    tc.tile_pool(name="pa_dr", bufs=2, space="DRAM") as pa_dr,
                ):
                    for nm, t_sb, tn_bf in (
                        ("x", x_sb, xn_bf),
                        ("g", g_sb, gn_bf),
                    ):
                        sq = pa_sb.tile([C, N], bf16, tag="sq", name="sq")
                        nc.vector.tensor_tensor(
                            out=sq, in0=t_sb, in1=t_sb, op=ALU.mult
                        )
                        rinv = pa_sb.tile([1, N], f32, tag="rinv", name="rinv")
                        for h in range(2):
                            n2 = pa_ps.tile([1, N // 2], f32, tag="n2", name="n2")
                            for j in range(4):
                                lo = j * 512
                                nc.tensor.matmul(
                                    n2[:, lo : lo + 512],
                                    ones_bf,
                                    sq[:, h * (N // 2) + lo : h * (N // 2) + lo + 512],
                                    start=True,
                                    stop=True,
                                )
                            half = pa_sb.tile([1, N // 2], f32, tag="nrm", name="half")
                            nc.scalar.activation(
                                out=half, in_=n2, func=ACTF.Sqrt,
                                bias=zero_b[0:1, :],
                            )
                            nc.vector.reciprocal(
                                out=rinv[:, h * (N // 2) : (h + 1) * (N // 2)],
                                in_=half,
                            )
                        r_d = pa_dr.tile([1, N], f32, tag="r_d", name="r_d")
                        nc.gpsimd.dma_start(out=r_d, in_=rinv)
                        rb = pa_sb1.tile([128, N], f32, tag="rb", name="rb")
                        nc.gpsimd.dma_start(out=rb, in_=_bcast(r_d, 128))
                        nc.vector.tensor_tensor(
                            out=tn_bf, in0=t_sb, in1=rb, op=ALU.mult
                        )

                    # ---- phase A2: XWT[n,o] = sum_c x[c,n] W[c,o] ----
                    if in_dtype == "bf16":
                        x_mm = x_sb
                    else:
                        x_mm = pa_sb1.tile([C, N], bf16, tag="xcast", name="x_mm")
                        nc.vector.tensor_copy(x_mm, x_sb)
                    for nj in range(NJ):
                        xwp = pa_ps2.tile([128, OUT], f32, tag="xwp", name="xwp")
                        nc.tensor.matmul(
                            xwp,
                            x_mm[:, nj * 128 : (nj + 1) * 128],
                            w_bf,
                            start=True,
                            stop=True,
                        )
                        nc.vector.tensor_copy(
                            xwt[:, nj * OUT : (nj + 1) * OUT], xwp
                        )

                # ---- phase B: fused attention ----
                with (
                    tc.tile_pool(name="stp", bufs=1, space="PSUM") as stp,
                    tc.tile_pool(name="o3p", bufs=1, space="PSUM") as o3p,
                    tc.tile_pool(name="rowp", bufs=1, space="PSUM") as rowp,
                    tc.tile_pool(name="ptp", bufs=2) as ptp,
                    tc.tile_pool(name="ep", bufs=2) as ep,
                    tc.tile_pool(name="epd", bufs=2, space="DRAM") as epd,
                ):
                    for mg in range(NMG):
                        m0 = mg * MGS
                        o3 = o3p.tile([OUT, MGS], f32, tag="o3", name="o3")
                        if rows_mode == "mm":
                            rows = rowp.tile([1, MGS], f32, tag="rows", name="rows")
                        else:
                            racc = ep.tile(
                                [1, 2 * MGS], f32, tag="racc", name="racc"
                            )
                            nc.vector.memset(racc, 0.0)
                        for njg in range(NJ // 2):
                            # S^T for two n-chunks side by side -> one exp
                            st = stp.tile([128, 2 * MGS], f32, tag="st", name="st")
                            for njl in range(2):
                                nj = 2 * njg + njl
                                for h in range(2):
                                    nc.tensor.matmul(
                                        st[:, njl * MGS + h * 512 : njl * MGS + (h + 1) * 512],
                                        xn_bf[:, nj * 128 : (nj + 1) * 128],
                                        gn_bf[:, m0 + h * 512 : m0 + (h + 1) * 512],
                                        start=True,
                                        stop=True,
                                    )
                            pt = ptp.tile([128, 2 * MGS], bf16, tag="pt", name="pt")
                            nc.scalar.activation(
                                out=pt, in_=st, func=ACTF.Exp, bias=zero_b
                            )
                            for njl in range(2):
                                nj = 2 * njg + njl
                                for h in range(2):
                                    sl = slice(h * 512, (h + 1) * 512)
                                    nc.tensor.matmul(
                                        o3[:, sl],
                                        xwt[:, nj * OUT : (nj + 1) * OUT],
                                        pt[:, njl * MGS + h * 512 : njl * MGS + (h + 1) * 512],
                                        start=(nj == 0),
                                        stop=(nj == NJ - 1),
                                    )
                            if rows_mode == "mm":
                                for njl in range(2):
                                    nj = 2 * njg + njl
                                    for h in range(2):
                                        sl = slice(h * 512, (h + 1) * 512)
                                        nc.tensor.matmul(
                                            rows[:, sl],
                                            ones_bf,
                                            pt[:, njl * MGS + h * 512 : njl * MGS + (h + 1) * 512],
                                            start=(nj == 0),
                                            stop=(nj == NJ - 1),
                                        )
                            else:
                                rpart = ptp.tile(
                                    [1, 2 * MGS], f32, tag="rpart", name="rpart"
                                )
                                nc.gpsimd.tensor_reduce(
                                    out=rpart, in_=pt,
                                    axis=mybir.AxisListType.C, op=ALU.add,
                                )
                                nc.vector.tensor_tensor(
                                    out=racc, in0=racc, in1=rpart, op=ALU.add,
                                )
                        # epilogue: LeakyReLU, /rowsum, *A -> bf16 delta out
                        rr = ep.tile([1, MGS], f32, tag="rr", name="rr")
                        if rows_mode == "mm":
                            nc.vector.reciprocal(out=rr, in_=rows)
                        else:
                            rfold = ep.tile([1, MGS], f32, tag="rfold", name="rfold")
                            nc.vector.tensor_tensor(
                                out=rfold, in0=racc[:, 0:MGS],
                                in1=racc[:, MGS : 2 * MGS], op=ALU.add,
                            )
                            nc.vector.reciprocal(out=rr, in_=rfold)
                        rr_d = epd.tile([1, MGS], f32, tag="rr_d", name="rr_d")
                        nc.gpsimd.dma_start(out=rr_d, in_=rr)
                        rrb = ep.tile([128, MGS], f32, tag="rrb", name="rrb")
                        nc.gpsimd.dma_start(out=rrb, in_=_bcast(rr_d, 128))
                        rra = ep.tile([128, MGS], f32, tag="rra", name="rra")
                        nc.vector.tensor_scalar(
                            out=rra, in0=rrb, scalar1=a_sc, scalar2=None,
                            op0=ALU.mult,
                        )
                        t1 = ep.tile([OUT, MGS], f32, tag="t1", name="t1")
                        nc.vector.tensor_scalar(
                            out=t1, in0=o3, scalar1=NEG_SLOPE, scalar2=None,
                            op0=ALU.mult,
                        )
                        z = ep.tile([OUT, MGS], f32, tag="z", name="z")
                        nc.vector.tensor_tensor(out=z, in0=o3, in1=t1, op=ALU.max)
                        yt = ep.tile([OUT, MGS], bf16, tag="yt", name="yt")
                        nc.vector.tensor_tensor(out=yt, in0=z, in1=rra, op=ALU.mult)
                        nc.gpsimd.dma_start(out=y_d[:, m0 : m0 + MGS], in_=yt)
    return nc


_nc_cache: dict = {}


def kernel(input, target_g, weight, gamma, beta, running_mean, running_var):
    import ml_dtypes

    from concourse.bass_utils import run_bass_kernel_spmd

    if "nc" not in _nc_cache:
        _nc_cache["nc"] = build_nc(repeats=1)
    nc = _nc_cache["nc"]

    x16 = np.ascontiguousarray(
        np.asarray(input, dtype=np.float32).astype(ml_dtypes.bfloat16)
    )
    g16 = np.ascontiguousarray(
        np.asarray(target_g, dtype=np.float32).astype(ml_dtypes.bfloat16)
    )
    weight = np.ascontiguousarray(np.asarray(weight, dtype=np.float32))
    gamma = np.asarray(gamma, dtype=np.float32).reshape(OUT)
    beta = np.asarray(beta, dtype=np.float32).reshape(OUT)
    mean = np.asarray(running_mean, dtype=np.float32).reshape(OUT)
    var = np.asarray(running_var, dtype=np.float32).reshape(OUT)

    # BN folded: y = z2 * A + Bc,  A applied on-device, Bc added on host
    a_sc = (gamma / np.sqrt(var + EPS_BN)).astype(np.float32)
    b_sc = (beta - mean * a_sc).astype(np.float32)

    in_maps = [
        {
            "x": x16[b],
            "g": g16[b],
            "w": weight,
            "a_sc": a_sc.reshape(OUT, 1),
        }
        for b in range(B)
    ]
    res = run_bass_kernel_spmd(nc, in_maps, core_ids=list(range(B)))
    y = np.stack([res.results[b]["y"] for b in range(B)]).astype(np.float32)
    return y + b_sc[None, :, None]

